# revision 1
# baseline (speedup 1.0000x reference)
"""Block-sparse matmul + bias + relu on 8 Trainium2 NeuronCores.

Strategy (data-parallel over batch):
  - Shard x along batch: 8 cores x 512 rows. w_blocks/bias replicated.
  - Per core, compute out^T = sum_blocks w_ij^T-style per-block matmuls with
    the PE in 32x32 tiling mode:
      * x^T resident in SBUF as [128, 32, 512]: input block i lives at
        partitions 32*(i%4) .. 32*(i%4)+31, free tile i//4.
      * each nonzero block (i,j) is one matmul: lhsT = w_block [K=32, M=32],
        rhs = x^T block i [32, 512], accumulated into PSUM at partition strip
        32*(j%4) of bank (i%4): tile_position=(32*(i%4), 32*(j%4)).
      * output block-cols processed in quads (4 cols -> 4 strips x 4 banks),
        16 PE tiles run concurrently.
  - Per quad combine: DVE sums bank pairs, GPSIMD sums the pair results,
    ACT applies bias + relu, DMA out^T tile to DRAM.
  - Host: transpose/cast prep (bf16 feeds the PE; fp32 accumulate in PSUM).
"""

import os

import numpy as np
import ml_dtypes

import concourse.bass as bass
import concourse.tile as tile
from concourse import mybir
from concourse.bass_utils import run_bass_kernel_spmd

LAST_RESULTS = None  # test-only: BassKernelResults of the last run

BS = 32
KB = 128
NB = 128
BATCH = 4096
NCORES = 8
BC = BATCH // NCORES          # 512 batch rows per core
NQ = NB // 4                  # 32 quads of output block-cols
if os.environ.get("BASS_KERNEL_F32R"):
    IN_DT = mybir.dt.float32r
    IN_NP = np.float32
else:
    IN_DT = mybir.dt.bfloat16
    IN_NP = ml_dtypes.bfloat16
if os.environ.get("BASS_KERNEL_OUT_BF16"):
    OUT_DT = mybir.dt.bfloat16
    OUT_NP = ml_dtypes.bfloat16
else:
    OUT_DT = mybir.dt.float32
    OUT_NP = np.float32

_CACHE = {}


def _build_schedule(row_idx, col_idx):
    """Schedule: per quad, round-robin emission over the 16 (strip, rowgrp)
    FIFOs. Returns (sched, S) where sched[q] is a list of
    (r, c, t, slot, start, stop) and S is the per-strip slot count in w image.
    Dummy (zero-weight) entries have slot == -1... they get real slots in the
    zero-padded region; we give them slot index with block = None marker via
    t=0 and a dedicated zero slot per strip.
    """
    nnz = len(row_idx)
    # FIFOs[q][c][r] -> list of block ids
    fifos = [[[[] for _ in range(4)] for _ in range(4)] for _ in range(NQ)]
    for n in range(nnz):
        i = int(row_idx[n]); j = int(col_idx[n])
        fifos[j // 4][j % 4][i % 4].append(n)

    slot_ctr = [0, 0, 0, 0]           # per row-group strip
    sched = []
    slot_of = {}                      # block id -> slot (in its strip)
    dummy_slots = []                  # (r, slot) zero-weight slots
    for q in range(NQ):
        # pad: every (c, r) needs >= 1 entry so PSUM region is defined
        entries = []                  # (r, c, t, block_or_None)
        maxlen = 0
        for c in range(4):
            for r in range(4):
                if not fifos[q][c][r]:
                    fifos[q][c][r].append(None)
                maxlen = max(maxlen, len(fifos[q][c][r]))
        emitted = []
        # r cycles fastest: consecutive MMs hit different row groups so the
        # PE can pull the next LDWEIGHTS ahead of in-flight MATMULs.
        for s in range(maxlen):
            for c in range(4):
                for r in range(4):
                    lst = fifos[q][c][r]
                    if s < len(lst):
                        n = lst[s]
                        if n is None:
                            slot = slot_ctr[r]; slot_ctr[r] += 1
                            dummy_slots.append((r, slot))
                            t = 0
                        else:
                            slot = slot_ctr[r]; slot_ctr[r] += 1
                            slot_of[n] = slot
                            t = int(row_idx[n]) // 4
                        emitted.append([r, c, t, slot, False, False])
        # start/stop flags per (bank r, strip c) accumulation region: the
        # PSUM has_written clear from start=True covers only the partitions
        # the matmul writes (one 32-partition strip, full bank width), so
        # every strip needs its own start.
        first_seen = set()
        for e in emitted:
            key = (e[0], e[1])
            if key not in first_seen:
                e[4] = True
                first_seen.add(key)
        last_idx = {}
        for k, e in enumerate(emitted):
            last_idx[(e[0], e[1])] = k
        for k in last_idx.values():
            emitted[k][5] = True
        sched.append([tuple(e) for e in emitted])
    S = max(slot_ctr)
    return sched, S, slot_of, dummy_slots


def _build_schedule_m128(row_idx, col_idx):
    """M=128 row-mode schedule: one matmul per (input block i, quad q) pair
    covering all four quad columns at once (lhsT [32, 128], zero-padded for
    missing cols). Output writes the full bank (partitions 0..127), so
    tile_position = (32*(i%4), 0).

    Returns (sched, S) with sched[q] = list of (r, t, slot, start, stop);
    slot indexes [32, 128] wide slots in the per-strip weight image, and
    wfill = list of (r, slot, c, n) for the image builder.
    """
    nnz = len(row_idx)
    by_iq = {}
    for n in range(nnz):
        i = int(row_idx[n]); j = int(col_idx[n])
        by_iq.setdefault((j // 4, i), []).append((j % 4, n))

    slot_ctr = [0, 0, 0, 0]
    sched = []
    wfill = []
    for q in range(NQ):
        fifos = [[] for _ in range(4)]        # per row group: list of i (or None)
        for i in range(KB):
            if (q, i) in by_iq:
                fifos[i % 4].append(i)
        for r in range(4):
            if not fifos[r]:
                fifos[r].append(None)
        emitted = []
        maxlen = max(len(f) for f in fifos)
        for s in range(maxlen):
            for r in range(4):
                if s < len(fifos[r]):
                    i = fifos[r][s]
                    slot = slot_ctr[r]; slot_ctr[r] += 1
                    if i is not None:
                        for (c, n) in by_iq[(q, i)]:
                            wfill.append((r, slot, c, n))
                        t = i // 4
                    else:
                        t = 0
                    emitted.append([r, t, slot, False, False])
        first_seen = set()
        for e in emitted:
            if e[0] not in first_seen:
                e[3] = True
                first_seen.add(e[0])
        last_idx = {}
        for k, e in enumerate(emitted):
            last_idx[e[0]] = k
        for k in last_idx.values():
            emitted[k][4] = True
        sched.append([tuple(e) for e in emitted])
    S = max(slot_ctr)
    return sched, S, wfill


_MULTIWAIT_OK = {"InstDMACopy", "InstUnconditionalBranch",
                 "InstConditionalBranch"}


def _legalize_waits(nc):
    """Engine ISA structs carry a single sync-wait slot; Tile can emit more.
    Offload excess waits onto same-engine NoOps inserted just before the
    instruction (per-engine stream order is the block list order)."""
    ctr = 0
    for f in nc.m.functions:
        for blk in f.blocks:
            out = []
            for inst in blk.instructions:
                si = inst.sync_info
                if (si is not None and si.on_wait and len(si.on_wait) > 1
                        and type(inst).__name__ == "InstDMACopy"):
                    # HWDGE lane sems are monotonic add-only counters; a
                    # DMA's wait on its own completion lane orders it against
                    # unrelated prior DMAs on that lane and is droppable.
                    own = {u.ant_name for u in (si.on_update or [])}
                    keep = [w for w in si.on_wait if w.ant_name not in own]
                    if len(keep) > 1:
                        raise RuntimeError(
                            f"DMA {inst.name} still has waits {keep}")
                    inst.sync_info = mybir.SyncInfo(on_wait=keep,
                                                    on_update=si.on_update)
                    out.append(inst)
                    continue
                if (si is not None and si.on_wait and len(si.on_wait) > 1
                        and type(inst).__name__ not in _MULTIWAIT_OK):
                    waits = list(si.on_wait)
                    for w in waits[:-1]:
                        nop = mybir.InstNoOp(name=f"waitnop-{ctr}")
                        ctr += 1
                        nop.engine = inst.engine
                        nop.sync_info = mybir.SyncInfo(on_wait=[w], on_update=[])
                        out.append(nop)
                    inst.sync_info = mybir.SyncInfo(on_wait=[waits[-1]],
                                                    on_update=si.on_update)
                out.append(inst)
            blk.instructions[:] = out


def _build_program(sched, S, repeat=1, loop_n=0, m128=False, dyn_loop=False):
    WSLOT = 128 if m128 else 32
    nc = bass.Bass("TRN2", target_bir_lowering=False, debug=False,
                   num_devices=NCORES)
    x_d = nc.dram_tensor("xt", [128, 32 * BC], IN_DT, kind="ExternalInput").ap()
    w_d = nc.dram_tensor("wim", [128, S * WSLOT], IN_DT, kind="ExternalInput").ap()
    b_d = nc.dram_tensor("bias", [128, 32], mybir.dt.float32,
                         kind="ExternalInput").ap()
    o_d = nc.dram_tensor("outT", [NQ, 128, BC], OUT_DT, kind="ExternalOutput").ap()
    ln_d = None
    if dyn_loop:
        ln_d = nc.dram_tensor("loopn", [1, 1], mybir.dt.uint32,
                              kind="ExternalInput").ap()

    import contextlib

    with tile.TileContext(nc) as tc:
        if dyn_loop:
            tmp = nc.alloc_registers("loopn_tmp", mybir.ALL_ENGINES)
            nc.regs_load(tmp, ln_d[0:1, 0:1])
            loop_end = nc.snap(tmp, donate=True, min_val=0, max_val=1 << 20)
            loop_cm = tc.For_i(0, loop_end, 1)
        elif loop_n:
            loop_cm = tc.For_i(0, loop_n, 1)
        else:
            loop_cm = contextlib.nullcontext()
        with tc.tile_pool(name="const", bufs=1) as cpool, \
             tc.tile_pool(name="work", bufs=3) as wpool, \
             tc.tile_pool(name="psum", bufs=2, space="PSUM") as ppool, \
             loop_cm:
            xt = cpool.tile([128, 32 * BC], IN_DT)
            wt = cpool.tile([128, S * WSLOT], IN_DT)
            bt = cpool.tile([128, 32], mybir.dt.float32)
            nc.sync.dma_start(bt[:], b_d[:])
            # x: chunked DMA (16 x 1MB)
            xch = (32 * BC) // 16
            for k in range(16):
                nc.sync.dma_start(xt[:, k * xch:(k + 1) * xch],
                                  x_d[:, k * xch:(k + 1) * xch])
            # w: chunked DMA in slot order so early quads unblock early
            wch = 8 if not m128 else 16
            wstep = -(-S // wch) * WSLOT
            for k in range(wch):
                lo = k * wstep
                hi = min(S * WSLOT, lo + wstep)
                if lo >= hi:
                    continue
                nc.sync.dma_start(wt[:, lo:hi], w_d[:, lo:hi])

            for rep in range(repeat):
              for q in range(NQ):
                acc = [ppool.tile([128, BC], mybir.dt.float32, tag=f"acc{r}",
                                  name=f"acc{r}_q{q}_p{rep}")
                       for r in range(4)]
                if m128:
                    for (r, t, slot, start, stop) in sched[q]:
                        nc.tensor.matmul(
                            out=acc[r][:, :],
                            lhsT=wt[32 * r:32 * r + 32,
                                    slot * 128:(slot + 1) * 128],
                            rhs=xt[32 * r:32 * r + 32, t * BC:(t + 1) * BC],
                            start=start, stop=stop,
                            tile_position=(32 * r, 0),
                            skip_group_check=True,
                        )
                else:
                    for (r, c, t, slot, start, stop) in sched[q]:
                        nc.tensor.matmul(
                            out=acc[r][32 * c:32 * c + 32, :],
                            lhsT=wt[32 * r:32 * r + 32,
                                    slot * 32:(slot + 1) * 32],
                            rhs=xt[32 * r:32 * r + 32, t * BC:(t + 1) * BC],
                            start=start, stop=stop,
                            tile_position=(32 * r, 32 * c),
                            skip_group_check=True,
                        )
                e0 = wpool.tile([128, BC], mybir.dt.float32, tag="e0")
                e2 = wpool.tile([128, BC], mybir.dt.float32, tag="e2")
                s1 = wpool.tile([128, BC], mybir.dt.float32, tag="s1")
                s2 = wpool.tile([128, BC], mybir.dt.float32, tag="s2")
                s3 = wpool.tile([128, BC], mybir.dt.float32, tag="s3")
                ot = wpool.tile([128, BC], OUT_DT, tag="ot")
                nc.scalar.copy(e0[:], acc[0][:])
                nc.scalar.copy(e2[:], acc[2][:])
                nc.vector.tensor_add(s1[:], acc[1][:], e0[:])
                nc.vector.tensor_add(s2[:], acc[3][:], e2[:])
                nc.gpsimd.tensor_add(s3[:], s1[:], s2[:])
                nc.gpsimd.tensor_scalar(ot[:], s3[:], bt[:, q:q + 1], 0.0,
                                        mybir.AluOpType.add,
                                        mybir.AluOpType.max)
                nc.sync.dma_start(o_d[q], ot[:])
    _legalize_waits(nc)
    return nc


def _prep_inputs_m128(x, w_blocks, bias, row_idx, col_idx, wfill, S):
    xb = x.astype(IN_NP).reshape(BATCH, 32, 4, 32)
    xt_all = np.ascontiguousarray(xb.transpose(2, 3, 1, 0)).reshape(128, 32, BATCH)
    xts = [np.ascontiguousarray(xt_all[:, :, c * BC:(c + 1) * BC]
                                ).reshape(128, 32 * BC) for c in range(NCORES)]
    bim = np.ascontiguousarray(
        bias.astype(np.float32).reshape(32, 4, 32).transpose(1, 2, 0)
    ).reshape(128, 32)
    wim = np.zeros((128, S * 128), dtype=IN_NP)
    wb = w_blocks.astype(IN_NP)
    for (r, slot, c, n) in wfill:
        wim[32 * r:32 * r + 32, 128 * slot + 32 * c:128 * slot + 32 * c + 32] \
            = wb[n]
    return xts, wim, bim


def _prep_inputs(x, w_blocks, bias, row_idx, col_idx, slot_of, dummy_slots, S):
    nnz = len(row_idx)
    # x^T images per core: [128, 32, BC] -> block i at partitions 32*(i%4),
    # free tile i//4.  x[b, 32*(4t+r)+p] -> xt[32r+p, t, b]
    xb = x.astype(IN_NP).reshape(BATCH, 32, 4, 32)        # b, t, r, p
    xt_all = np.ascontiguousarray(xb.transpose(2, 3, 1, 0))  # r, p, t, b
    xt_all = xt_all.reshape(128, 32, BATCH)
    xts = [np.ascontiguousarray(xt_all[:, :, c * BC:(c + 1) * BC]
                                ).reshape(128, 32 * BC) for c in range(NCORES)]
    # w image [128, S*32]
    wim = np.zeros((128, S * 32), dtype=IN_NP)
    wb = w_blocks.astype(IN_NP)
    for n in range(nnz):
        r = int(row_idx[n]) % 4
        s = slot_of[n]
        wim[32 * r:32 * r + 32, 32 * s:32 * s + 32] = wb[n]
    # dummy slots already zero
    bim = np.ascontiguousarray(
        bias.astype(np.float32).reshape(32, 4, 32).transpose(1, 2, 0)
    ).reshape(128, 32)
    return xts, wim, bim


def kernel(x, w_blocks, bias, row_idx, col_idx):
    repeat = int(os.environ.get("BASS_KERNEL_REPEAT", "1"))
    m128 = bool(os.environ.get("BASS_KERNEL_M128"))
    key = (row_idx.tobytes(), col_idx.tobytes(), repeat, m128)
    if key not in _CACHE:
        if m128:
            sched, S, wfill = _build_schedule_m128(row_idx, col_idx)
            aux = wfill
        else:
            sched, S, slot_of, dummy_slots = _build_schedule(row_idx, col_idx)
            aux = (slot_of, dummy_slots)
        nc = _build_program(sched, S, repeat=repeat, m128=m128)
        _CACHE[key] = (nc, S, aux)
    nc, S, aux = _CACHE[key]

    if m128:
        xts, wim, bim = _prep_inputs_m128(x, w_blocks, bias, row_idx, col_idx,
                                          aux, S)
    else:
        slot_of, dummy_slots = aux
        xts, wim, bim = _prep_inputs(x, w_blocks, bias, row_idx, col_idx,
                                     slot_of, dummy_slots, S)
    in_maps = [{"xt": xts[c], "wim": wim, "bias": bim} for c in range(NCORES)]
    trace = bool(os.environ.get("BASS_KERNEL_TRACE"))
    res = run_bass_kernel_spmd(nc, in_maps, list(range(NCORES)), trace=trace)
    global LAST_RESULTS
    LAST_RESULTS = res

    out = np.empty((BATCH, NB * BS), dtype=np.float32)
    for c in range(NCORES):
        outT = res.results[c]["outT"].reshape(NB * BS, BC)
        out[c * BC:(c + 1) * BC, :] = outT.T.astype(np.float32)
    return out



# revision 10
# speedup vs baseline: 2.6328x; 2.6328x over previous
"""Block-sparse matmul + bias + relu on 8 Trainium2 NeuronCores.

Strategy (data-parallel over batch):
  - Shard x along batch: 8 cores x 512 rows. w_blocks/bias replicated.
  - Per core, compute out^T: x^T resident in SBUF as [128, 32, 512] (input
    block i at partition strip 32*(i%4), free tile i//4); each nonzero block
    (i,j) is one 32x32-tile matmul (lhsT = w block, rhs = x^T block strip,
    N=512 batch).
  - Single-bank accumulation: all 16 (strip, col) cells of an output quad
    accumulate into ONE PSUM bank by separating row groups in time.  Groups
    of 4 quads run 4 rounds with a Latin-square phase rotation (quad a does
    row group (k+a)%4 in round k), so the 16 PE tiles stay concurrent across
    quads while each quad's bank sees only one row group at a time.
  - Epilogue per quad: one ACT op relu(acc + bias) -> bf16, then DMA out^T.
    No DVE/GPSIMD work at all.
  - Post-passes: offload multi-waits onto NoOps; thin per-MM semaphore
    increments to just the waited-on counts.
  - Host: transpose/cast prep (bf16 feeds the PE; fp32 accumulate in PSUM).
"""

import os

import numpy as np
import ml_dtypes

import concourse.bass as bass
import concourse.tile as tile
from concourse import mybir
from concourse.bass_utils import run_bass_kernel_spmd

LAST_RESULTS = None  # test-only: BassKernelResults of the last run

BS = 32
KB = 128
NB = 128
BATCH = 4096
NCORES = 8
BC = BATCH // NCORES          # 512 batch rows per core
NQ = NB // 4                  # 32 quads of output block-cols
NG = NQ // 4                  # 8 groups of 4 quads
if os.environ.get("BASS_KERNEL_F32R"):
    IN_DT = mybir.dt.float32r
    IN_NP = np.float32
else:
    IN_DT = mybir.dt.bfloat16
    IN_NP = ml_dtypes.bfloat16
if os.environ.get("BASS_KERNEL_OUT_F32"):
    OUT_DT = mybir.dt.float32
    OUT_NP = np.float32
else:
    OUT_DT = mybir.dt.bfloat16
    OUT_NP = ml_dtypes.bfloat16

_CACHE = {}


def _build_schedule(row_idx, col_idx):
    """Latin-square phased schedule.

    Returns (sched, S, slot_of, dummy_slots):
      sched[g] = list of (q, p, c, t, slot, start, stop) in emission order
        for quad group g (quads 4g..4g+3); p is the row group (i%4) and the
        PE tile is (32p, 32c); all 16 cells of quad q accumulate into ONE
        PSUM bank.
      S = per-strip slot count of the weight image.
    """
    nnz = len(row_idx)
    cells = [[[[] for _ in range(4)] for _ in range(4)] for _ in range(NQ)]
    for n in range(nnz):
        i = int(row_idx[n]); j = int(col_idx[n])
        cells[j // 4][i % 4][j % 4].append(n)

    slot_ctr = [0, 0, 0, 0]
    slot_of = {}
    dummy_slots = []
    sched = []
    bounds = []   # (wait_on_gidx, at_gidx): at-MM must wait completion of
                  # wait_on-MM (same quad's previous round last MM) so two
                  # row-group tiles never stream into one PSUM bank at once
    gidx = 0
    for g in range(NG):
        quads = [4 * g + a for a in range(4)]
        # every (q, c) region needs >= 1 MM so the PSUM strip is defined
        for q in quads:
            for c in range(4):
                if not any(cells[q][p][c] for p in range(4)):
                    cells[q][0][c].append(None)
        ent = []
        last_of_quad = {}   # quad -> gidx of its last MM emitted so far
        for k in range(4):
            for a, q in enumerate(quads):
                p = (k + a) % 4
                maxd = max(len(cells[q][p][c]) for c in range(4))
                firstq = True
                for s in range(maxd):
                    for c in range(4):
                        lst = cells[q][p][c]
                        if s < len(lst):
                            n = lst[s]
                            slot = slot_ctr[p]
                            slot_ctr[p] += 1
                            if n is None:
                                dummy_slots.append((p, slot))
                                t = 0
                            else:
                                slot_of[n] = slot
                                t = int(row_idx[n]) // 4
                            if firstq and q in last_of_quad:
                                bounds.append((last_of_quad[q], gidx))
                            firstq = False
                            last_of_quad[q] = gidx
                            ent.append([q, p, c, t, slot, False, False])
                            gidx += 1
        # start/stop per (q, c) region in emission order
        first = {}
        last = {}
        for idx, e in enumerate(ent):
            key = (e[0], e[2])
            if key not in first:
                first[key] = idx
            last[key] = idx
        for idx in first.values():
            ent[idx][5] = True
        for idx in last.values():
            ent[idx][6] = True
        sched.append([tuple(e) for e in ent])
    S = max(slot_ctr)
    return sched, S, slot_of, dummy_slots, bounds


_MULTIWAIT_OK = {"InstDMACopy", "InstUnconditionalBranch",
                 "InstConditionalBranch"}


def _legalize_waits(nc):
    """Engine ISA structs carry a single sync-wait slot; Tile can emit more.
    Offload excess waits onto same-engine NoOps inserted just before the
    instruction (per-engine stream order is the block list order)."""
    ctr = 0
    for f in nc.m.functions:
        for blk in f.blocks:
            out = []
            for inst in blk.instructions:
                si = inst.sync_info
                if (si is not None and si.on_wait and len(si.on_wait) > 1
                        and type(inst).__name__ == "InstDMACopy"):
                    # HWDGE lane sems are monotonic add-only counters; a
                    # DMA's wait on its own completion lane orders it against
                    # unrelated prior DMAs on that lane and is droppable.
                    own = {u.ant_name for u in (si.on_update or [])}
                    keep = [w for w in si.on_wait if w.ant_name not in own]
                    if len(keep) > 1:
                        raise RuntimeError(
                            f"DMA {inst.name} still has waits {keep}")
                    inst.sync_info = mybir.SyncInfo(on_wait=keep,
                                                    on_update=si.on_update)
                    out.append(inst)
                    continue
                if (si is not None and si.on_wait and len(si.on_wait) > 1
                        and type(inst).__name__ not in _MULTIWAIT_OK):
                    waits = list(si.on_wait)
                    for w in waits[:-1]:
                        nop = mybir.InstNoOp(name=f"waitnop-{ctr}")
                        ctr += 1
                        nop.engine = inst.engine
                        nop.sync_info = mybir.SyncInfo(on_wait=[w], on_update=[])
                        out.append(nop)
                    inst.sync_info = mybir.SyncInfo(on_wait=[waits[-1]],
                                                    on_update=si.on_update)
                out.append(inst)
            blk.instructions[:] = out


def _inject_round_waits(nc, bounds, n_mm_per_rep):
    """Make each quad's round-k first MM wait for the completion (semaphore
    count) of that quad's round-(k-1) last MM.  An MM's PE-lane increment
    fires after its PSUM drain, so this guarantees two row-group tiles never
    stream into the same PSUM bank simultaneously.  Must run before
    _legalize_waits (multi-wait fixup) and _thin_pe_incs (which preserves all
    waited-on counts)."""
    for f in nc.m.functions:
        for blk in f.blocks:
            mms = [i for i in blk.instructions
                   if type(i).__name__ == "InstMatmult"
                   and getattr(i, "engine", None) == mybir.EngineType.PE]
            if len(mms) < n_mm_per_rep:
                continue
            assert len(mms) % n_mm_per_rep == 0, (len(mms), n_mm_per_rep)
            # PE lane sem template from any MM's update
            tmpl = None
            for i in mms:
                si = i.sync_info
                for u in (si.on_update or []) if si else []:
                    if u.ant_name.startswith("PE_"):
                        tmpl = u
                        break
                if tmpl:
                    break
            assert tmpl is not None
            nrep = len(mms) // n_mm_per_rep
            for rep in range(nrep):
                off = rep * n_mm_per_rep
                for (wait_on, at) in bounds:
                    inst = mms[off + at]
                    si = inst.sync_info or mybir.SyncInfo(on_wait=[],
                                                          on_update=[])
                    w = mybir.SyncWait(sync_type="semaphore", id=tmpl.id,
                                       ant_name=tmpl.ant_name,
                                       wait_mode="sem-ge-imm",
                                       wait_value=off + wait_on + 1)
                    inst.sync_info = mybir.SyncInfo(
                        on_wait=list(si.on_wait or []) + [w],
                        on_update=si.on_update)


def _thin_pe_incs(nc, lane_prefix="PE_"):
    """Per-MM semaphore increments serialize on the PE EVT_SEM port.  Since
    the PE completes instructions in pc order, the counting semaphore only
    needs to move at values someone actually waits on: keep an increment
    exactly at each waited cumulative count (as a sem-add-imm jump covering
    the dropped increments before it) plus the final one.  Every wait keeps
    its original literal value; each waited value is reached when the SAME
    instruction (or a later one) completes, so ordering is preserved."""
    waited = {}
    for f in nc.m.functions:
        for blk in f.blocks:
            for inst in blk.instructions:
                si = inst.sync_info
                if not si:
                    continue
                for w in (si.on_wait or []):
                    if w.ant_name.startswith(lane_prefix):
                        assert w.wait_mode == "sem-ge-imm", w
                        waited.setdefault(w.ant_name, set()).add(w.wait_value)
    for f in nc.m.functions:
        for blk in f.blocks:
            incs = {}
            for inst in blk.instructions:
                si = inst.sync_info
                if not si or not si.on_update:
                    continue
                if type(inst).__name__ == "InstEventSemaphore":
                    continue  # protocol add/sub bookkeeping: leave alone
                for u in si.on_update:
                    if (u.ant_name.startswith(lane_prefix)
                            and u.update_mode == "sem-inc"):
                        incs.setdefault(u.ant_name, []).append((inst, u))
            for sem, lst in incs.items():
                vset = waited.get(sem, set())
                cum = 0
                pending = 0
                for k, (inst, u) in enumerate(lst):
                    cum += 1
                    pending += 1
                    if cum in vset or k == len(lst) - 1:
                        u.update_mode = "sem-add-imm"
                        u.update_value = pending
                        pending = 0
                    else:
                        si = inst.sync_info
                        keep = [x for x in si.on_update if x is not u]
                        inst.sync_info = mybir.SyncInfo(on_wait=si.on_wait,
                                                        on_update=keep)
                assert pending == 0


def _build_program(sched, S, repeat=1, loop_n=0, bounds=None):
    nc = bass.Bass("TRN2", target_bir_lowering=False, debug=False,
                   num_devices=NCORES)
    x_d = nc.dram_tensor("xt", [128, 32 * BC], IN_DT, kind="ExternalInput").ap()
    w_d = nc.dram_tensor("wim", [128, S * 32], IN_DT, kind="ExternalInput").ap()
    b_d = nc.dram_tensor("bias", [128, 32], mybir.dt.float32,
                         kind="ExternalInput").ap()
    o_d = nc.dram_tensor("outT", [NQ, 128, BC], OUT_DT, kind="ExternalOutput").ap()

    import contextlib

    with tile.TileContext(nc) as tc:
        loop_cm = tc.For_i(0, loop_n, 1) if loop_n else contextlib.nullcontext()
        with tc.tile_pool(name="const", bufs=2) as cpool, \
             tc.tile_pool(name="work", bufs=4) as wpool, \
             tc.tile_pool(name="psum", bufs=2, space="PSUM") as ppool, \
             loop_cm:
            xt = cpool.tile([128, 32 * BC], IN_DT, tag="xt")
            wt = cpool.tile([128, S * 32], IN_DT, tag="wt")
            bt = cpool.tile([128, 32], mybir.dt.float32, tag="bt")
            nc.sync.dma_start(bt[:], b_d[:])
            # x: chunked DMA (16 x 1MB)
            xch = (32 * BC) // 16
            for k in range(16):
                nc.sync.dma_start(xt[:, k * xch:(k + 1) * xch],
                                  x_d[:, k * xch:(k + 1) * xch])
            # w: chunked DMA in slot order so early groups unblock early
            wch = 8
            wstep = -(-S // wch) * 32
            for k in range(wch):
                lo = k * wstep
                hi = min(S * 32, lo + wstep)
                if lo >= hi:
                    continue
                nc.sync.dma_start(wt[:, lo:hi], w_d[:, lo:hi])

            for rep in range(repeat):
              for g in range(NG):
                acc = [ppool.tile([128, BC], mybir.dt.float32, tag=f"acc{a}",
                                  name=f"acc{a}_g{g}_p{rep}")
                       for a in range(4)]
                for (q, p, c, t, slot, start, stop) in sched[g]:
                    nc.tensor.matmul(
                        out=acc[q % 4][32 * c:32 * c + 32, :],
                        lhsT=wt[32 * p:32 * p + 32,
                                slot * 32:(slot + 1) * 32],
                        rhs=xt[32 * p:32 * p + 32, t * BC:(t + 1) * BC],
                        start=start, stop=stop,
                        tile_position=(32 * p, 32 * c),
                        skip_group_check=True,
                    )
                for a in range(4):
                    q = 4 * g + a
                    ot = wpool.tile([128, BC], OUT_DT, tag=f"ot{a}",
                                    name=f"ot{a}_g{g}_p{rep}")
                    nc.scalar.activation(
                        ot[:], acc[a][:], mybir.ActivationFunctionType.Relu,
                        bias=bt[:, q:q + 1], scale=1.0)
                    nc.sync.dma_start(o_d[q], ot[:])
    if bounds:
        n_mm = sum(len(g) for g in sched)
        _inject_round_waits(nc, bounds, n_mm)
    _legalize_waits(nc)
    if not os.environ.get("BASS_KERNEL_NO_THIN"):
        _thin_pe_incs(nc)
    return nc


def _prep_inputs(x, w_blocks, bias, row_idx, col_idx, slot_of, dummy_slots, S):
    nnz = len(row_idx)
    # x^T images per core: [128, 32, BC] -> block i at partitions 32*(i%4),
    # free tile i//4.  x[b, 32*(4t+r)+p] -> xt[32r+p, t, b]
    xb = x.astype(IN_NP).reshape(BATCH, 32, 4, 32)        # b, t, r, p
    xt_all = np.ascontiguousarray(xb.transpose(2, 3, 1, 0))  # r, p, t, b
    xt_all = xt_all.reshape(128, 32, BATCH)
    xts = [np.ascontiguousarray(xt_all[:, :, c * BC:(c + 1) * BC]
                                ).reshape(128, 32 * BC) for c in range(NCORES)]
    # w image [128, S*32]: block n at partition strip 32*(row%4), slot
    wim = np.zeros((128, S * 32), dtype=IN_NP)
    wb = w_blocks.astype(IN_NP)
    for n in range(nnz):
        r = int(row_idx[n]) % 4
        s = slot_of[n]
        wim[32 * r:32 * r + 32, 32 * s:32 * s + 32] = wb[n]
    # dummy slots already zero
    bim = np.ascontiguousarray(
        bias.astype(np.float32).reshape(32, 4, 32).transpose(1, 2, 0)
    ).reshape(128, 32)
    return xts, wim, bim


def kernel(x, w_blocks, bias, row_idx, col_idx):
    repeat = int(os.environ.get("BASS_KERNEL_REPEAT", "1"))
    key = (row_idx.tobytes(), col_idx.tobytes(), repeat)
    if key not in _CACHE:
        sched, S, slot_of, dummy_slots, bounds = _build_schedule(row_idx,
                                                                 col_idx)
        nc = _build_program(sched, S, repeat=repeat, bounds=bounds)
        _CACHE[key] = (nc, S, (slot_of, dummy_slots))
    nc, S, aux = _CACHE[key]

    slot_of, dummy_slots = aux
    xts, wim, bim = _prep_inputs(x, w_blocks, bias, row_idx, col_idx,
                                 slot_of, dummy_slots, S)
    in_maps = [{"xt": xts[c], "wim": wim, "bias": bim} for c in range(NCORES)]
    trace = bool(os.environ.get("BASS_KERNEL_TRACE"))
    res = run_bass_kernel_spmd(nc, in_maps, list(range(NCORES)), trace=trace)
    global LAST_RESULTS
    LAST_RESULTS = res

    out = np.empty((BATCH, NB * BS), dtype=np.float32)
    for c in range(NCORES):
        outT = res.results[c]["outT"].reshape(NB * BS, BC)
        out[c * BC:(c + 1) * BC, :] = outT.T.astype(np.float32)
    return out
